# revision 1
# baseline (speedup 1.0000x reference)
"""Gumbel-softmax vector quantizer on 8 Trainium2 NeuronCores.

Sharding: data-parallel over tokens (B*T = 16384 -> 2048 tokens/core).
W and codebook replicated. Per core, per 128-token tile, per group g:
  logits_g = x @ W[:, gV:(g+1)V]  (fp16 hi/lo 3-matmul split == fp32 accurate)
  s = logits + gumbel             (DVE, PSUM+SBUF)
  argmax via DVE max/max_index; probs via ACT exp(s/tau) with fused sum
  quantized rows via GPSIMD indirect-DMA gather from the codebook.
"""
import sys
sys.path.insert(0, '/opt/trn_rl_repo')
import numpy as np

import concourse.bass as bass
import concourse.bacc as bacc
import concourse.tile as tile
from concourse import mybir
from concourse.bass_utils import run_bass_kernel_spmd

F32 = mybir.dt.float32
F16 = mybir.dt.float16
I32 = mybir.dt.int32
U32 = mybir.dt.uint32

N_CORES = 8
B, T, DIN = 4, 4096, 1024
G, V, D = 4, 1024, 128
TAU = 2.0
TOK = B * T                  # 16384
TPC = TOK // N_CORES         # 2048 tokens per core
NT = TPC // 128              # 16 tiles of 128 tokens
KT = DIN // 128              # 8 k-tiles
NN = V // 512                # 2 n-slices of 512 per group

_prog = None


def _build():
    nc = bacc.Bacc("TRN2", target_bir_lowering=False, debug=False,
                   num_devices=N_CORES)
    xt_h = nc.dram_tensor("xt_h", [NT, DIN, 128], F16, kind="ExternalInput").ap()
    xt_l = nc.dram_tensor("xt_l", [NT, DIN, 128], F16, kind="ExternalInput").ap()
    w_h = nc.dram_tensor("w_h", [DIN, G * V], F16, kind="ExternalInput").ap()
    w_l = nc.dram_tensor("w_l", [DIN, G * V], F16, kind="ExternalInput").ap()
    gum = nc.dram_tensor("gum", [TPC, G * V], F32, kind="ExternalInput").ap()
    np_t = nc.dram_tensor("np_t", [128, NT], F32, kind="ExternalInput").ap()
    cb = nc.dram_tensor("cb", [G * V, D], F32, kind="ExternalInput").ap()

    o_ids = nc.dram_tensor("o_ids", [TPC, G], I32, kind="ExternalOutput").ap()
    o_qnt = nc.dram_tensor("o_qnt", [TPC, G * D], F32, kind="ExternalOutput").ap()
    o_prb = nc.dram_tensor("o_prb", [TPC, G * V], F32, kind="ExternalOutput").ap()

    with tile.TileContext(nc) as tc:
        with tc.tile_pool(name="wpool", bufs=1) as wpool, \
             tc.tile_pool(name="xp", bufs=2) as xp, \
             tc.tile_pool(name="gp", bufs=3) as gp, \
             tc.tile_pool(name="sp", bufs=2) as sp, \
             tc.tile_pool(name="op", bufs=2) as op, \
             tc.tile_pool(name="mp", bufs=4) as mp, \
             tc.tile_pool(name="ps", bufs=2, space="PSUM") as ps:

            wh_t = wpool.tile([128, KT, G * V], F16, tag="wh")
            nc.sync.dma_start(wh_t[:], w_h.rearrange("(k p) n -> p k n", p=128))
            wl_t = wpool.tile([128, KT, G * V], F16, tag="wl")
            nc.sync.dma_start(wl_t[:], w_l.rearrange("(k p) n -> p k n", p=128))
            npd_t = wpool.tile([128, NT], F32, tag="npd")
            nc.sync.dma_start(npd_t[:], np_t[:])
            npm1_t = wpool.tile([128, NT], F32, tag="npm1")
            nc.vector.tensor_scalar(out=npm1_t[:], in0=npd_t[:], scalar1=-1.0,
                                    scalar2=None, op0=mybir.AluOpType.add)

            for t in range(NT):
                xh_t = xp.tile([128, KT, 128], F16, tag="xh")
                nc.sync.dma_start(xh_t[:],
                                  xt_h[t].rearrange("(k p) j -> p k j", p=128))
                xl_t = xp.tile([128, KT, 128], F16, tag="xl")
                nc.sync.dma_start(xl_t[:],
                                  xt_l[t].rearrange("(k p) j -> p k j", p=128))
                qnt_t = op.tile([128, G * D], F32, tag="qnt")
                ids_t = mp.tile([128, G], I32, tag="ids")

                for g in range(G):
                    gum_t = gp.tile([128, V], F32, tag="gum")
                    nc.sync.dma_start(
                        gum_t[:],
                        gum[t * 128:(t + 1) * 128, g * V:(g + 1) * V])

                    acc = ps.tile([128, V], F32, tag="acc")
                    for k in range(KT):
                        for xo, wo, last in ((xh_t, wh_t, False),
                                             (xh_t, wl_t, False),
                                             (xl_t, wh_t, True)):
                            for n in range(NN):
                                nsl = slice(g * V + n * 512, g * V + (n + 1) * 512)
                                nc.tensor.matmul(
                                    acc[:, n * 512:(n + 1) * 512],
                                    xo[:, k, :], wo[:, k, nsl],
                                    start=(k == 0 and xo is xh_t and wo is wh_t),
                                    stop=(k == KT - 1 and last))

                    s_t = sp.tile([128, V], F32, tag="s")
                    nc.vector.tensor_tensor(out=s_t[:], in0=acc[:], in1=gum_t[:],
                                            op=mybir.AluOpType.add)

                    mx8 = mp.tile([128, 8], F32, tag="mx8")
                    nc.vector.max(mx8[:], s_t[:])
                    idx8 = mp.tile([128, 8], U32, tag="idx8")
                    nc.vector.max_index(idx8[:], mx8[:], s_t[:])

                    e_t = sp.tile([128, V], F32, tag="e")
                    z_t = mp.tile([128, 1], F32, tag="z")
                    nc.scalar.activation(e_t[:], s_t[:],
                                         mybir.ActivationFunctionType.Exp,
                                         scale=1.0 / TAU, accum_out=z_t[:])
                    rz_t = mp.tile([128, 1], F32, tag="rz")
                    nc.vector.reciprocal(rz_t[:], z_t[:])
                    sc_t = mp.tile([128, 1], F32, tag="sc")
                    nc.vector.tensor_tensor(out=sc_t[:], in0=rz_t[:],
                                            in1=npd_t[:, t:t + 1],
                                            op=mybir.AluOpType.mult)
                    prb_t = sp.tile([128, V], F32, tag="prb")
                    nc.vector.tensor_scalar_mul(prb_t[:], e_t[:], sc_t[:])
                    nc.sync.dma_start(
                        o_prb[t * 128:(t + 1) * 128, g * V:(g + 1) * V],
                        prb_t[:])

                    # ids_out = idx * nonpad + (nonpad - 1)
                    nc.vector.scalar_tensor_tensor(
                        out=ids_t[:, g:g + 1], in0=idx8[:, 0:1],
                        scalar=npd_t[:, t:t + 1], in1=npm1_t[:, t:t + 1],
                        op0=mybir.AluOpType.mult, op1=mybir.AluOpType.add)

                    # gather codebook rows: idxg = idx + g*V
                    idxg = mp.tile([128, 1], I32, tag="idxg")
                    nc.vector.tensor_scalar(out=idxg[:], in0=idx8[:, 0:1],
                                            scalar1=float(g * V), scalar2=None,
                                            op0=mybir.AluOpType.add)
                    nc.gpsimd.indirect_dma_start(
                        out=qnt_t[:, g * D:(g + 1) * D], out_offset=None,
                        in_=cb[:],
                        in_offset=bass.IndirectOffsetOnAxis(ap=idxg[:, :1],
                                                            axis=0))

                nc.sync.dma_start(o_ids[t * 128:(t + 1) * 128, :], ids_t[:])
                qm_t = op.tile([128, G * D], F32, tag="qm")
                nc.vector.tensor_scalar_mul(qm_t[:], qnt_t[:], npd_t[:, t:t + 1])
                nc.sync.dma_start(o_qnt[t * 128:(t + 1) * 128, :], qm_t[:])

    nc.compile()
    return nc


def kernel(inputs, paddings, gumbel, W, b, codebook):
    global _prog
    if _prog is None:
        _prog = _build()
    nc = _prog

    x = np.ascontiguousarray(inputs.reshape(TOK, DIN))
    gum = gumbel.reshape(TOK, G * V)
    if b.any():
        gum = gum + b.reshape(1, G * V).astype(np.float32)
    gum = np.ascontiguousarray(gum)
    pad = paddings.reshape(TOK)
    nonpad = (1 - pad).astype(np.float32)

    w_h = W.astype(np.float16)
    w_l = (W - w_h.astype(np.float32)).astype(np.float16)
    cb_flat = np.ascontiguousarray(codebook.reshape(G * V, D).astype(np.float32))

    in_maps = []
    for c in range(N_CORES):
        sl = slice(c * TPC, (c + 1) * TPC)
        xc = x[sl].reshape(NT, 128, DIN).transpose(0, 2, 1)  # (NT, DIN, 128)
        xh = np.ascontiguousarray(xc.astype(np.float16))
        xl = np.ascontiguousarray((xc - xh.astype(np.float32)).astype(np.float16))
        np_tile = np.ascontiguousarray(nonpad[sl].reshape(NT, 128).T)
        in_maps.append(dict(xt_h=xh, xt_l=xl, w_h=w_h, w_l=w_l,
                            gum=gum[sl], np_t=np_tile, cb=cb_flat))

    res = run_bass_kernel_spmd(nc, in_maps, list(range(N_CORES)))
    ids = np.concatenate([r["o_ids"] for r in res.results], axis=0)
    qnt = np.concatenate([r["o_qnt"] for r in res.results], axis=0)
    prb = np.concatenate([r["o_prb"] for r in res.results], axis=0)
    return (ids.reshape(B, T, G).astype(np.int32),
            qnt.reshape(B, T, G * D),
            prb.reshape(B, T, G, V))


# revision 6
# speedup vs baseline: 25836.5417x; 25836.5417x over previous
"""Gumbel-softmax vector quantizer on 8 Trainium2 NeuronCores.

Sharding: data-parallel over tokens (B*T = 16384 -> 2048 tokens/core).
W and codebook replicated. Per core, per 128-token tile, per group g:
  logits_g = x @ W[:, gV:(g+1)V]  (fp16 hi/lo 3-matmul split == fp32 accurate)
  s = logits + gumbel             (DVE, PSUM+SBUF)
  argmax via DVE max/max_index; probs via ACT exp(s/tau) with fused sum
  quantized rows via GPSIMD indirect-DMA gather from the codebook.
"""
import sys
sys.path.insert(0, '/opt/trn_rl_repo')
import numpy as np

import concourse.bass as bass
import concourse.bacc as bacc
import concourse.tile as tile
from concourse import mybir
from concourse.bass_utils import run_bass_kernel_spmd

F32 = mybir.dt.float32
F16 = mybir.dt.float16
I32 = mybir.dt.int32
U32 = mybir.dt.uint32

N_CORES = 8
B, T, DIN = 4, 4096, 1024
G, V, D = 4, 1024, 128
TAU = 2.0
TOK = B * T                  # 16384
TPC = TOK // N_CORES         # 2048 tokens per core
NT = TPC // 128              # 16 tiles of 128 tokens
KT = DIN // 128              # 8 k-tiles
NN = V // 512                # 2 n-slices of 512 per group

_prog = None


def _build(ps_bufs=2, gp_bufs=3, sp_bufs=2, xp_bufs=2, split_w=False, passes=3):
    nc = bacc.Bacc("TRN2", target_bir_lowering=False, debug=False,
                   num_devices=N_CORES)
    xt_h = nc.dram_tensor("xt_h", [NT, DIN, 128], F16, kind="ExternalInput").ap()
    xt_l = nc.dram_tensor("xt_l", [NT, DIN, 128], F16, kind="ExternalInput").ap()
    w_h = nc.dram_tensor("w_h", [DIN, G * V], F16, kind="ExternalInput").ap()
    w_l = nc.dram_tensor("w_l", [DIN, G * V], F16, kind="ExternalInput").ap()
    gum = nc.dram_tensor("gum", [TPC, G * V], F32, kind="ExternalInput").ap()
    np_t = nc.dram_tensor("np_t", [128, NT], F32, kind="ExternalInput").ap()
    cb = nc.dram_tensor("cb", [G * V, D], F32, kind="ExternalInput").ap()

    o_ids = nc.dram_tensor("o_ids", [TPC, G], I32, kind="ExternalOutput").ap()
    o_qnt = nc.dram_tensor("o_qnt", [TPC, G * D], F32, kind="ExternalOutput").ap()
    o_prb = nc.dram_tensor("o_prb", [TPC, G * V], F32, kind="ExternalOutput").ap()

    with tile.TileContext(nc) as tc:
        with tc.tile_pool(name="wpool", bufs=1) as wpool, \
             tc.tile_pool(name="xp", bufs=xp_bufs) as xp, \
             tc.tile_pool(name="gp", bufs=gp_bufs) as gp, \
             tc.tile_pool(name="sp", bufs=sp_bufs) as sp, \
             tc.tile_pool(name="op", bufs=2) as op, \
             tc.tile_pool(name="mp", bufs=4) as mp, \
             tc.tile_pool(name="ps", bufs=ps_bufs, space="PSUM") as ps:

            wh_t = wpool.tile([128, KT, G * V], F16, tag="wh")
            wl_t = wpool.tile([128, KT, G * V], F16, tag="wl")
            if split_w:
                # group-column chunks, in consumption order, so group-0
                # matmuls start after 1/4 of W arrives
                for g in range(G):
                    gsl = slice(g * V, (g + 1) * V)
                    nc.sync.dma_start(
                        wh_t[:, :, gsl],
                        w_h[:, gsl].rearrange("(k p) n -> p k n", p=128))
                    nc.sync.dma_start(
                        wl_t[:, :, gsl],
                        w_l[:, gsl].rearrange("(k p) n -> p k n", p=128))
            else:
                nc.sync.dma_start(wh_t[:],
                                  w_h.rearrange("(k p) n -> p k n", p=128))
                nc.sync.dma_start(wl_t[:],
                                  w_l.rearrange("(k p) n -> p k n", p=128))
            npd_t = wpool.tile([128, NT], F32, tag="npd")
            nc.sync.dma_start(npd_t[:], np_t[:])
            npm1_t = wpool.tile([128, NT], F32, tag="npm1")
            nc.vector.tensor_scalar(out=npm1_t[:], in0=npd_t[:], scalar1=-1.0,
                                    scalar2=None, op0=mybir.AluOpType.add)

            for t in range(NT):
                xh_t = xp.tile([128, KT, 128], F16, tag="xh")
                nc.sync.dma_start(xh_t[:],
                                  xt_h[t].rearrange("(k p) j -> p k j", p=128))
                xl_t = xp.tile([128, KT, 128], F16, tag="xl")
                nc.sync.dma_start(xl_t[:],
                                  xt_l[t].rearrange("(k p) j -> p k j", p=128))
                qnt_t = op.tile([128, G * D], F32, tag="qnt")
                ids_t = mp.tile([128, G], I32, tag="ids")

                for g in range(G):
                    gum_t = gp.tile([128, V], F32, tag="gum")
                    nc.sync.dma_start(
                        gum_t[:],
                        gum[t * 128:(t + 1) * 128, g * V:(g + 1) * V])

                    acc = ps.tile([128, V], F32, tag="acc")
                    for k in range(KT):
                        plist = ((xh_t, wh_t, False), (xh_t, wl_t, False),
                                 (xl_t, wh_t, True))[:passes]
                        plist = plist[:-1] + ((plist[-1][0], plist[-1][1], True),)
                        for xo, wo, last in plist:
                            for n in range(NN):
                                nsl = slice(g * V + n * 512, g * V + (n + 1) * 512)
                                nc.tensor.matmul(
                                    acc[:, n * 512:(n + 1) * 512],
                                    xo[:, k, :], wo[:, k, nsl],
                                    start=(k == 0 and xo is xh_t and wo is wh_t),
                                    stop=(k == KT - 1 and last))

                    s_t = sp.tile([128, V], F32, tag="s")
                    nc.vector.tensor_tensor(out=s_t[:], in0=acc[:], in1=gum_t[:],
                                            op=mybir.AluOpType.add)

                    mx8 = mp.tile([128, 8], F32, tag="mx8")
                    nc.vector.max(mx8[:], s_t[:])
                    idx8 = mp.tile([128, 8], U32, tag="idx8")
                    nc.vector.max_index(idx8[:], mx8[:], s_t[:])

                    e_t = sp.tile([128, V], F32, tag="e")
                    z_t = mp.tile([128, 1], F32, tag="z")
                    nc.scalar.activation(e_t[:], s_t[:],
                                         mybir.ActivationFunctionType.Exp,
                                         scale=1.0 / TAU, accum_out=z_t[:])
                    rz_t = mp.tile([128, 1], F32, tag="rz")
                    nc.vector.reciprocal(rz_t[:], z_t[:])
                    sc_t = mp.tile([128, 1], F32, tag="sc")
                    nc.vector.tensor_tensor(out=sc_t[:], in0=rz_t[:],
                                            in1=npd_t[:, t:t + 1],
                                            op=mybir.AluOpType.mult)
                    prb_t = sp.tile([128, V], F32, tag="prb")
                    nc.vector.tensor_scalar_mul(prb_t[:], e_t[:], sc_t[:])
                    nc.sync.dma_start(
                        o_prb[t * 128:(t + 1) * 128, g * V:(g + 1) * V],
                        prb_t[:])

                    # ids_out = idx * nonpad + (nonpad - 1)
                    nc.vector.scalar_tensor_tensor(
                        out=ids_t[:, g:g + 1], in0=idx8[:, 0:1],
                        scalar=npd_t[:, t:t + 1], in1=npm1_t[:, t:t + 1],
                        op0=mybir.AluOpType.mult, op1=mybir.AluOpType.add)

                    # gather codebook rows: idxg = idx + g*V
                    idxg = mp.tile([128, 1], I32, tag="idxg")
                    nc.vector.tensor_scalar(out=idxg[:], in0=idx8[:, 0:1],
                                            scalar1=float(g * V), scalar2=None,
                                            op0=mybir.AluOpType.add)
                    nc.gpsimd.indirect_dma_start(
                        out=qnt_t[:, g * D:(g + 1) * D], out_offset=None,
                        in_=cb[:],
                        in_offset=bass.IndirectOffsetOnAxis(ap=idxg[:, :1],
                                                            axis=0))

                nc.sync.dma_start(o_ids[t * 128:(t + 1) * 128, :], ids_t[:])
                qm_t = op.tile([128, G * D], F32, tag="qm")
                nc.vector.tensor_scalar_mul(qm_t[:], qnt_t[:], npd_t[:, t:t + 1])
                nc.sync.dma_start(o_qnt[t * 128:(t + 1) * 128, :], qm_t[:])

    nc.compile()
    return nc


def kernel(inputs, paddings, gumbel, W, b, codebook):
    global _prog
    if _prog is None:
        _prog = _build()
    nc = _prog

    x = np.ascontiguousarray(inputs.reshape(TOK, DIN))
    gum = gumbel.reshape(TOK, G * V)
    if b.any():
        gum = gum + b.reshape(1, G * V).astype(np.float32)
    gum = np.ascontiguousarray(gum)
    pad = paddings.reshape(TOK)
    nonpad = (1 - pad).astype(np.float32)

    w_h = W.astype(np.float16)
    w_l = (W - w_h.astype(np.float32)).astype(np.float16)
    cb_flat = np.ascontiguousarray(codebook.reshape(G * V, D).astype(np.float32))

    in_maps = []
    for c in range(N_CORES):
        sl = slice(c * TPC, (c + 1) * TPC)
        xc = x[sl].reshape(NT, 128, DIN).transpose(0, 2, 1)  # (NT, DIN, 128)
        xh = np.ascontiguousarray(xc.astype(np.float16))
        xl = np.ascontiguousarray((xc - xh.astype(np.float32)).astype(np.float16))
        np_tile = np.ascontiguousarray(nonpad[sl].reshape(NT, 128).T)
        in_maps.append(dict(xt_h=xh, xt_l=xl, w_h=w_h, w_l=w_l,
                            gum=gum[sl], np_t=np_tile, cb=cb_flat))

    res = run_bass_kernel_spmd(nc, in_maps, list(range(N_CORES)))
    ids = np.concatenate([r["o_ids"] for r in res.results], axis=0)
    qnt = np.concatenate([r["o_qnt"] for r in res.results], axis=0)
    prb = np.concatenate([r["o_prb"] for r in res.results], axis=0)
    return (ids.reshape(B, T, G).astype(np.int32),
            qnt.reshape(B, T, G * D),
            prb.reshape(B, T, G, V))
